# revision 125
# baseline (speedup 1.0000x reference)
"""Causal self-attention (B=2, T=2048, C=1024, H=16) on 8 TRN2 NeuronCores.

Sharding: batch x heads — core c owns batch c//4 and heads {4g..4g+3}, g=c%4.
Each core computes a partial o_proj output for its batch [T, C]; the host sums
the 4 partials per batch and adds o_b.

Per-core pipeline (fp32 PSUM accumulate):
  x ships as fp8 main + residual; qkv = 3 DoubleRow fp8 passes
  (x8@W8 + x8@R + xr8@W8, R in e5m2 — bf16-level accuracy at 0.75x bf16)
  q evacuates as packed fp8 main+residual (q8, qr8); k as fp8 main only
  S^T[j,i] = (q8+qr8).k8 in ONE fp8 DoubleRow pass per j-block — the pair
  slots carry (q8, qr8) against a stride-0-broadcast k8, so S costs half of
  bf16 and q is exactly corrected (k stays fp8-rounded, ~1.8% on weights)
  P^T = exp(S^T/8)  — ACT activation for most groups; the first off-diagonal
  group of chunks 1-3 instead uses a one-op DVE Schraudolph (int16 affine ->
  bf16 bits, ~1.8% rms) to offload the saturated ACT engine
  diagonal blocks masked by a 128x128 triu mask (DVE)
  O_aug[i, 65] += P^T.T @ vaug    (PE flipped PV: P stationary, vaug moving,
                                   col 64 = softmax denominator for free)
  attO[i, hd] = O * recip(denom)  (DVE per-partition scalar — no broadcast)
  attT = xbar-DMA transpose(attO) (back to [hd, t] for o_proj)
  y[t, e] = attT^T @ ow           (PE, 2 k-steps over 256 head-dims)

Schedule: 4 query chunks; heads overlap at boundaries (each head's last
PV-flush+normalize is issued after the next head's first S/exp); qkv/oproj/
transposes are woven in as PE filler against the ACT exp stream; the tail
oprojs run on the freed sc/pv PSUM banks with ACT/DVE-alternating evacs.
NOTE: program order IS dependency order for the tile framework — a read
emitted before its writer races (no sync is inserted), so unit ordering
below is correctness-relevant, not just performance tuning.
"""

import os
import numpy as np

WARMUP = int(os.environ.get("K_WARMUP", "1"))
OVERLAP = int(os.environ.get("K_OVERLAP", "1"))
KBCAST = int(os.environ.get("K_KBCAST", "1"))
# chunks whose g0/g1 exps run as a Schraudolph bit-trick on the DVE (~1.8%
# rms on those attention weights) to offload the saturated ACT engine
# "ic:ng" pairs: first ng off-diagonal groups of chunk ic use the DVE exp
SCHRAU = os.environ.get("K_SCHRAU", "1:1,2:1,3:1")
SCHRAU_NG = {}
for s in SCHRAU.split(","):
    if s:
        a, b = s.split(":")
        SCHRAU_NG[int(a)] = int(b)

B = 2
T = 2048
C = 1024
H = 16
DH = 64
NCORES = 8
HPC = 4                     # heads per core
CPB = 4                     # cores per batch
TB = T // 128               # 16 t-blocks
KB = C // 128               # 8 contraction blocks for qkv
NCH = T // 512              # 4 i-chunks

_nc_cache = None


def _interleave(primary, filler):
    """Emit primary units with filler units woven in (filler spread evenly).

    A primary entry may be a tuple (fn, allow_fill_after=False) to keep the
    slot after it filler-free (used after each head's last S group so the
    next head's first S isn't queued behind filler matmuls on the PE)."""
    np_, nf = len(primary), len(filler)
    fi = 0
    for i, u in enumerate(primary):
        fn, allow = u if isinstance(u, tuple) else (u, True)
        fn()
        want = int(round((i + 1) * nf / max(np_, 1)))
        while fi < want and allow:
            filler[fi]()
            fi += 1
    while fi < nf:
        filler[fi]()
        fi += 1


def build_bass(dbg=False):
    import concourse.bass as bass
    import concourse.bacc as bacc
    import concourse.tile as tile
    import concourse.mybir as mybir

    F32 = mybir.dt.float32
    BF16 = mybir.dt.bfloat16
    FP8 = mybir.dt.float8e4
    FP8E5 = mybir.dt.float8e5
    DR = mybir.MatmulPerfMode.DoubleRow
    Exp = mybir.ActivationFunctionType.Exp
    Mult = mybir.AluOpType.mult

    nc = bacc.Bacc("TRN2", target_bir_lowering=False, debug=False)

    # x and qkv weights ship as fp8 main + residual (DoubleRow matmuls at
    # half cycles/row; the residual passes recover bf16-level accuracy)
    x8_d = nc.dram_tensor("x8", [C, T], FP8, kind="ExternalInput")
    xr8_d = nc.dram_tensor("xr8", [C, T], FP8, kind="ExternalInput")
    ident_d = nc.dram_tensor("ident", [128, 128], BF16, kind="ExternalInput")
    w_d = nc.dram_tensor("w", [128, KB * 768], FP8, kind="ExternalInput")
    # weight residual in e5m2: wide exponent range lets pass 2 consume x8
    # directly (no scaled x8s copy of x needed at all)
    r_d = nc.dram_tensor("r", [128, KB * 768], FP8E5, kind="ExternalInput")
    bias_d = nc.dram_tensor("bqkv", [128, 6], F32, kind="ExternalInput")
    ow_d = nc.dram_tensor("ow", [128, 2 * C], BF16, kind="ExternalInput")
    tri_d = nc.dram_tensor("tri", [128, 128], BF16, kind="ExternalInput")
    y_d = nc.dram_tensor("y", [T, C], BF16, kind="ExternalOutput")
    if dbg:
        dbg_d = {nm: nc.dram_tensor(f"dbg_{nm}", [128, 2 * T], BF16,
                                    kind="ExternalOutput")
                 for nm in ("qT", "kT", "vT", "attT")}
        dbg_d["vaug0"] = nc.dram_tensor("dbg_vaug0", [128, TB * 65], BF16,
                                        kind="ExternalOutput")
        dbg_d["attO"] = nc.dram_tensor("dbg_attO", [128, TB * 256], BF16,
                                       kind="ExternalOutput")

    with tile.TileContext(nc) as tc:
        with (
            tc.tile_pool(name="const", bufs=1) as constp,
            tc.tile_pool(name="xT", bufs=1) as xtp,
            tc.tile_pool(name="qkv", bufs=1) as qkvp,
            tc.tile_pool(name="vaug", bufs=1) as vaugp,
            tc.tile_pool(name="vstage", bufs=6) as vstagep,
            tc.tile_pool(name="pT", bufs=8) as ptp,
            tc.tile_pool(name="att", bufs=1) as attp,
            tc.tile_pool(name="recip", bufs=4) as recipp,
            tc.tile_pool(name="yout", bufs=6) as youtp,
            tc.tile_pool(name="ps", bufs=1, space="PSUM") as ps,
        ):
            w_sb = constp.tile([128, KB * 768], FP8)
            r_sb = constp.tile([128, KB * 768], FP8E5)
            ow_sb = constp.tile([128, 2 * C], BF16)
            bias_sb = constp.tile([128, 6], F32)
            tri_sb = constp.tile([128, 128], BF16)
            ident_sb = constp.tile([128, 128], BF16)
            warm_sb = constp.tile([128, 128], BF16)
            # w layout: [p, (mt, kq, pair, 128)] — DoubleRow pairs adjacent
            # kb blocks; per-m-tile DMA so qkv(0) starts as soon as its own
            # weights land
            w3 = w_sb[:].rearrange("p (mt kq pr c) -> p mt kq pr c",
                                   kq=KB // 2, pr=2, c=128)
            r3 = r_sb[:].rearrange("p (mt kq pr c) -> p mt kq pr c",
                                   kq=KB // 2, pr=2, c=128)

            def wdma_unit(mts, res=False):
                def run():
                    sb, d = (r_sb, r_d) if res else (w_sb, w_d)
                    nc.sync.dma_start(
                        sb[:, mts[0] * 1024:mts[1] * 1024],
                        d[:, mts[0] * 1024:mts[1] * 1024])
                return run

            def const_dma_unit():
                # ident needed from phase 0 (stationary of the mask matmul)
                def run():
                    nc.sync.dma_start(tri_sb[:], tri_d[:])
                    nc.sync.dma_start(ident_sb[:], ident_d[:])
                return run

            def late_const_dma_unit():
                # ow first used by oproj (phase 2)
                def run():
                    nc.sync.dma_start(ow_sb[:], ow_d[:])
                return run

            xT8 = xtp.tile([128, KB * T], FP8, name="x8")    # [c, (kb, t)]
            xTr8 = xtp.tile([128, KB * T], FP8, name="xr8")
            xparts = [(xT8, x8_d), (xTr8, xr8_d)]
            xviews = [t[:].rearrange("p (kb t) -> p kb t", t=T)
                      for t, _ in xparts]
            # q: fp8 main+residual packed for DoubleRow S: partition
            # dh + 64*(h%2); free (hh, which, t) = (h//2)*2T + which*T + t.
            # S = (q8 + qr8) . k8 in ONE DR pass (q fully corrected, k at
            # fp8 rounding) — halves S matmul time vs bf16.
            q8p = qkvp.tile([128, 4 * T], FP8, name="q8p")
            # k: fp8 main only [dh, (hh, t)]; the DR pair dim is supplied by
            # a stride-0 broadcast (both slots read the same k8)
            k8p = qkvp.tile([128, 2 * T], FP8, name="k8p")
            vT = qkvp.tile([128, 2 * T], BF16, name="vT")
            qpv = q8p[:].rearrange("p (hh w t) -> p hh w t", hh=2, w=2, t=T)
            if not KBCAST:
                k8d = qkvp.tile([128, 4 * T], FP8, name="k8d")
                kdup = k8d[:].rearrange("p (hh w t) -> p hh w t", hh=2, w=2, t=T)
            # vaug[h]: [t, (tb, 65)] — v natural + ones column (denominator)
            vaug = [vaugp.tile([128, TB * 65], BF16, name=f"vaug{h}")
                    for h in range(HPC)]
            # attO: [t, (tb, 4 heads * 64)] normalized attention out
            attO = attp.tile([128, TB * 256], BF16, name="attO")
            attOv = attO[:].rearrange("p (tb d) -> p tb d", d=256)
            # attT: [hd, (tb, hp, 128)] transposed back for o_proj
            attT = attp.tile([128, 2 * T], BF16, name="attT")

            def xdma_unit(tc_, kbs=None, parts=range(2)):
                def run():
                    for pi in parts:
                        d = xparts[pi][1]
                        dv = xviews[pi]
                        src = d.rearrange("(kb p) t -> p kb t", p=128)
                        if kbs is not None:   # kb sub-range (startup split)
                            nc.sync.dma_start(
                                dv[:, kbs[0]:kbs[1], tc_ * 512:(tc_ + 1) * 512],
                                src[:, kbs[0]:kbs[1], tc_ * 512:(tc_ + 1) * 512])
                        else:
                            nc.sync.dma_start(
                                dv[:, :, tc_ * 512:(tc_ + 1) * 512],
                                src[:, :, tc_ * 512:(tc_ + 1) * 512])
                return run

            def qkv_unit(tc_, mt, resid_pool=False, evac_act=False):
                """One 512-wide t-chunk of one 128-col m-tile (q0,q1,k0,k1,v0,v1)."""
                kind = (0, 1, 2, 2, 0, 1)[mt]    # 0=q, 1=k, 2=v
                hp = (0, 0, 0, 1, 1, 1)[mt]

                def run():
                    pt = ps.tile([128, 512], F32, name="psqkv", tag="b1", bufs=2)
                    # 3 DoubleRow passes: x8@W8 + x8@R + xr8@W8 (R in
                    # e5m2 so no scaled copy of x is needed)
                    passes = ((w3, xviews[0]), (r3, xviews[0]), (w3, xviews[1]))
                    for pi, (wv, xv) in enumerate(passes):
                        for kq in range(KB // 2):
                            nc.tensor.matmul(
                                pt[:],
                                wv[:, mt, kq],
                                xv[:, 2 * kq:2 * kq + 2,
                                   tc_ * 512:(tc_ + 1) * 512],
                                start=(pi == 0 and kq == 0),
                                stop=(pi == 2 and kq == KB // 2 - 1),
                                perf_mode=DR,
                            )
                    sl = slice(tc_ * 512, (tc_ + 1) * 512)
                    if kind == 2:
                        nc.vector.tensor_scalar_add(
                            vT[:, hp * T + tc_ * 512: hp * T + (tc_ + 1) * 512],
                            pt[:], bias_sb[:, mt:mt + 1])
                    elif kind == 1:
                        if KBCAST:
                            if evac_act:
                                nc.scalar.activation(
                                    k8p[:, hp * T + tc_ * 512:
                                        hp * T + (tc_ + 1) * 512],
                                    pt[:],
                                    mybir.ActivationFunctionType.Identity,
                                    bias=bias_sb[:, mt:mt + 1])
                            else:
                                nc.vector.tensor_scalar_add(
                                    k8p[:, hp * T + tc_ * 512:
                                        hp * T + (tc_ + 1) * 512],
                                    pt[:], bias_sb[:, mt:mt + 1])
                        else:
                            for wsl in range(2):
                                nc.vector.tensor_scalar_add(
                                    kdup[:, hp, wsl, tc_ * 512:(tc_ + 1) * 512],
                                    pt[:], bias_sb[:, mt:mt + 1])
                    else:
                        base = hp * 2 * T
                        m8 = q8p[:, base + tc_ * 512: base + (tc_ + 1) * 512]
                        if evac_act:
                            nc.scalar.activation(
                                m8, pt[:], mybir.ActivationFunctionType.Identity,
                                bias=bias_sb[:, mt:mt + 1])
                        else:
                            nc.vector.tensor_scalar_add(
                                m8, pt[:], bias_sb[:, mt:mt + 1])
                        # residual (q+b) - q8 -> fp8 (on Pool in the prologue
                        # so it overlaps the k evac on DVE)
                        eng = nc.gpsimd if resid_pool else nc.vector
                        eng.scalar_tensor_tensor(
                            q8p[:, base + T + tc_ * 512:
                                base + T + (tc_ + 1) * 512],
                            pt[:], bias_sb[:, mt:mt + 1], m8,
                            mybir.AluOpType.add, mybir.AluOpType.subtract)
                return run

            def vaug_unit(tc_, h):
                """xbar-transpose v rows for head h, t-chunk tc_, into vaug.

                The XBAR transpose only writes contiguous outputs, so it lands
                in a staging tile; a Pool copy fans it into the 65-stride
                augmented layout."""
                def run():
                    va = vaug[h][:].rearrange("p (tb d) -> p tb d", d=65)
                    if tc_ == 0:
                        nc.vector.memset(va[:, :, 64:65], 1.0)
                    vs = vstagep.tile([128, 4 * 64], BF16, name="vstage",
                                      tag="vs", bufs=6)
                    nc.sync.dma_start_transpose(
                        vs[:].rearrange("p (tb d) -> p tb d", d=64),
                        vT[(h % 2) * 64:(h % 2) * 64 + 64,
                           (h // 2) * T + tc_ * 512:(h // 2) * T + (tc_ + 1) * 512])
                    nc.gpsimd.tensor_copy(
                        va[:, tc_ * 4:(tc_ + 1) * 4, 0:64],
                        vs[:].rearrange("p (tb d) -> p tb d", d=64))
                return run

            def attn_units(ic, heads=range(HPC)):
                """Attention group-units for i-chunk ic (head-sequential)."""
                i0 = 512 * ic
                n_jb = 4 * (ic + 1)
                units = []
                state = {}

                def emit_pv(h, g):
                    """Mask + PV matmuls for group g (exp(g) already issued)."""
                    pv = state[h]
                    pt_sb = state[h, g]
                    for u in range(2):
                        jb = 2 * g + u
                        o = 128 * jb - i0
                        if o >= 0:   # diagonal block: causal mask
                            seg = pt_sb[:, u * 512 + o: u * 512 + o + 128]
                            nc.vector.tensor_tensor(seg, seg, tri_sb[:], Mult)
                        for ib in range(4):
                            if jb > 4 * ic + ib:
                                continue
                            # start=True clears has_written for the WHOLE
                            # bank, so only the first matmul into this pv
                            # tile may use it; the other ib slices start
                            # via overwrite-on-cleared-bits.
                            nc.tensor.matmul(
                                pv[:, ib * 65:(ib + 1) * 65],
                                pt_sb[:, u * 512 + ib * 128:
                                      u * 512 + (ib + 1) * 128],
                                vaug[h][:, jb * 65: jb * 65 + 65],
                                start=(jb == 0 and ib == 0),
                                stop=(jb == 4 * ic + ib),
                                skip_group_check=True,
                            )

                def group(h, g, hq, hh):
                    """S + exp for group g; PV lagged one group behind so the
                    PE never sits in its own FIFO waiting on exp(g)."""
                    if g == 0:
                        state[h] = ps.tile([128, 4 * 65], F32, name="pspv",
                                           tag="pv", bufs=2)
                    sps = ps.tile([128, 1024], F32, name="pssc",
                                  tag="sc", bufs=2)
                    for u in range(2):
                        jb = 2 * g + u
                        o = 128 * jb - i0
                        # trim to the causal span only on the trailing group
                        # (whose exp reads exactly the written spans); the
                        # other diagonal group is written in full so the full
                        # [0:1024] exp never reads uninitialized PSUM
                        lo = max(o, 0) if (2 * g + 1 == n_jb - 1) else 0
                        if KBCAST:
                            stat = (k8p[hq:hq + 64,
                                        hh * T + jb * 128:hh * T + jb * 128 + 128]
                                    .unsqueeze(1).broadcast_to((64, 2, 128)))
                        else:
                            stat = kdup[hq:hq + 64, hh, :,
                                        jb * 128:jb * 128 + 128]
                        nc.tensor.matmul(
                            sps[:, u * 512 + lo:(u + 1) * 512],
                            stat,
                            qpv[hq:hq + 64, hh, :, i0 + lo:i0 + 512],
                            start=True, stop=True,
                            perf_mode=DR,
                        )
                    pt_sb = ptp.tile([128, 1024], BF16, name="pt",
                                     tag="pt", bufs=8)
                    state[h, g] = pt_sb
                    if 2 * g + 1 == n_jb - 1:
                        # trailing diagonal pair: exp only the causal spans
                        nc.scalar.activation(pt_sb[:, 256:512],
                                             sps[:, 256:512], Exp, scale=0.125)
                        nc.scalar.activation(pt_sb[:, 896:1024],
                                             sps[:, 896:1024], Exp, scale=0.125)
                    elif g < SCHRAU_NG.get(ic, 0):
                        # off-diagonal group on DVE: bf16 bits of exp(S/8) via
                        # one int16 affine (Schraudolph), freeing ACT time
                        nc.vector.tensor_scalar(
                            pt_sb[:].bitcast(mybir.dt.int16), sps[:],
                            0.125 * 1.4426950408889634 * 128.0,
                            127.0 * 128.0 - 7.0,
                            mybir.AluOpType.mult, mybir.AluOpType.add)
                    else:
                        nc.scalar.activation(pt_sb[:], sps[:], Exp, scale=0.125)
                    if g > 0:
                        emit_pv(h, g - 1)

                def flush(h):
                    """Last PV group + normalize — split into its own unit so
                    the NEXT head's first S/exp is issued before it (the exp
                    stream never waits on the previous head's PV flush)."""
                    emit_pv(h, n_jb // 2 - 1)
                    pv = state[h]
                    rec = recipp.tile([128, 4], F32, name="rec", tag="rc")
                    pvv = pv[:].rearrange("p (ib d) -> p ib d", d=65)
                    nc.vector.reciprocal(
                        rec[:].rearrange("p (ib o) -> p ib o", o=1),
                        pvv[:, :, 64:65])
                    for ib in range(4):
                        tb = ic * 4 + ib
                        nc.vector.tensor_scalar_mul(
                            attOv[:, tb, h * 64:(h + 1) * 64],
                            pvv[:, ib, 0:64],
                            rec[:, ib:ib + 1])

                pending = None
                last_h = list(heads)[-1]
                for h in heads:
                    hq = (h % 2) * 64
                    hh = h // 2
                    for g in range(n_jb // 2):
                        fn = (lambda h=h, g=g, hq=hq, hh=hh:
                              group(h, g, hq, hh))
                        if g == n_jb // 2 - 1 and h != last_h:
                            # no filler after the last S group: the next
                            # head's first S must reach the PE immediately
                            units.append((fn, False))
                        else:
                            units.append(fn)
                        if OVERLAP and g == 0 and pending is not None:
                            units.append(pending)
                            pending = None
                    if not OVERLAP:
                        units.append(lambda h=h: flush(h))
                        pending = None
                    else:
                        pending = (lambda h=h: flush(h))
                if pending is not None:
                    units.append(pending)
                return units

            def oxpose_unit(ic):
                """One xbar call transposes the whole chunk's attO block."""
                def run():
                    nc.sync.dma_start_transpose(
                        attT[:, ic * 1024:(ic + 1) * 1024]
                        .rearrange("p (g t) -> p g t", t=128),
                        attO[:, ic * 1024:(ic + 1) * 1024])
                return run

            def oxpose_pe_unit(ic, hp, ibs=range(4), tag="b1"):
                """PE-transpose one head-pair's attO blocks of chunk ic —
                used on the last chunk where the xbar DMA round-trip would
                sit on the critical tail (runs as soon as that head pair's
                normalizes land)."""
                def run():
                    for ib in ibs:
                        tb = ic * 4 + ib
                        pt = ps.tile([128, 128], BF16, name="psxp", tag=tag,
                                     bufs=2)
                        nc.tensor.transpose(
                            pt[:], attOv[:, tb, hp * 128:(hp + 1) * 128],
                            ident_sb[:])
                        nc.vector.tensor_copy(
                            attT[:, (tb * 2 + hp) * 128:(tb * 2 + hp + 1) * 128],
                            pt[:])
                return run

            def oproj_unit(tb, act_share=False, split_dma=False, tail=False,
                           tail_dve=False):
                def run():
                    yo = youtp.tile([128, C], BF16, name="yo")
                    if tail:
                        # epilogue: attention is done, so the (wider) sc psum
                        # tiles are free — one 1024-wide psum breaks the b1
                        # double-buffer convoy; the evac is split ACT/DVE so
                        # the two halves run in parallel, and the y DMA is
                        # split across both DMA queues
                        pt2 = ps.tile([128, 1024], F32, name="pssc",
                                      tag="sc", bufs=2)
                        for ec in range(2):
                            for hp in range(2):
                                g = tb * 2 + hp
                                nc.tensor.matmul(
                                    pt2[:, ec * 512:(ec + 1) * 512],
                                    attT[:, g * 128:(g + 1) * 128],
                                    ow_sb[:, hp * C + ec * 512:
                                          hp * C + ec * 512 + 512],
                                    start=(hp == 0), stop=(hp == 1),
                                )
                        if tail_dve:
                            nc.vector.tensor_copy(yo[:], pt2[:])
                            nc.sync.dma_start(
                                y_d[tb * 128:(tb + 1) * 128, :], yo[:])
                        else:
                            nc.scalar.activation(
                                yo[:], pt2[:],
                                mybir.ActivationFunctionType.Copy)
                            nc.scalar.dma_start(
                                y_d[tb * 128:(tb + 1) * 128, :], yo[:])
                        return
                    for ec in range(2):
                        pt = ps.tile([128, 512], F32, name="psy", tag="b1",
                                     bufs=2)
                        for hp in range(2):
                            g = tb * 2 + hp
                            nc.tensor.matmul(
                                pt[:],
                                attT[:, g * 128:(g + 1) * 128],
                                ow_sb[:, hp * C + ec * 512:
                                      hp * C + ec * 512 + 512],
                                start=(hp == 0), stop=(hp == 1),
                            )
                        dst = yo[:, ec * 512:(ec + 1) * 512]
                        if act_share and ec == 1:
                            nc.scalar.activation(
                                dst, pt[:], mybir.ActivationFunctionType.Copy)
                        else:
                            nc.vector.tensor_copy(dst, pt[:])
                    if split_dma:   # epilogue: issue from the idle ACT queue
                        nc.scalar.dma_start(
                            y_d[tb * 128:(tb + 1) * 128, :], yo[:])
                    else:
                        nc.sync.dma_start(
                            y_d[tb * 128:(tb + 1) * 128, :], yo[:])
                return run

            # ---- schedule ----
            # Prologue feeds the hp0 heads of chunk 0 ASAP so ACT starts
            # early; after that each phase interleaves attention(ic) with
            # qkv(ic+1) and oproj(ic-1) so PE stays dense while ACT exps.
            # startup DMA order: smallest slices that unblock qkv(0, mt0/1)
            # first, everything else streamed behind them (each extra DMA
            # costs ~625ns on the HWDGE issue path, so only the startup is
            # finely sliced)
            nc.gpsimd.memset(warm_sb[:], 0.0)
            wdma_unit((0, 2))()
            xdma_unit(0, parts=(0,))()
            wdma_unit((0, 2), res=True)()
            xdma_unit(0, parts=(1,))()
            nc.sync.dma_start(bias_sb[:], bias_d[:])
            wdma_unit((2, 4))()
            wdma_unit((2, 4), res=True)()
            const_dma_unit()()
            wdma_unit((4, 6))()
            wdma_unit((4, 6), res=True)()
            xdma_unit(1)()
            xdma_unit(2)()
            # warm up the PE clock ramp during the initial DMA wait: cheap
            # matmuls on a memset tile into the sc psum tiles (no readers);
            # by the time real matmuls arrive the ramp is past the low gear
            warm_mov = warm_sb[:, 0:128].unsqueeze(1).broadcast_to((128, 4, 128))
            for _ in range(3 if WARMUP else 0):
                pwm = ps.tile([128, 1024], F32, name="pssc", tag="sc", bufs=2)
                nc.tensor.matmul(pwm[:, 0:512], warm_sb[:, 0:128],
                                 warm_mov, start=True, stop=True)

            # phase 0a: hand-sequenced so every vaug write precedes the first
            # PV that reads it (program order IS the dependency order for the
            # tile framework — a read emitted before its writer races)
            a0 = attn_units(0, heads=(0, 1))
            qkv_unit(0, 0)()
            qkv_unit(0, 1)()
            a0[0]()                  # h0 g0: S+exp only
            qkv_unit(0, 2)()
            vaug_unit(0, 0)()
            a0[1]()                  # h0 g1 (emits PV(h0, g0))
            vaug_unit(0, 1)()
            a0[2]()                  # h1 g0
            qkv_unit(0, 3)()
            a0[3]()                  # flush(h0)
            vaug_unit(0, 2)()
            a0[4]()                  # h1 g1
            qkv_unit(0, 4)()
            a0[5]()                  # flush(h1)
            vaug_unit(0, 3)()
            qkv_unit(0, 5)()
            _interleave(attn_units(0, heads=(2, 3)),
                        [qkv_unit(1, 2), qkv_unit(1, 3),
                         qkv_unit(1, 0, evac_act=True),
                         qkv_unit(1, 1, evac_act=True)] +
                        [vaug_unit(1, 0), vaug_unit(1, 1)])
            # phases 1-3: attention(ic) with oproj(ic-2) units spliced in right
            # after each head boundary (the boundary's normalize feeds the DVE
            # queue just ahead of that oproj's evacs — no cross-queue convoy).
            # v-projections go first in the filler so the vaug transposes
            # never block the SP DMA queue.
            # phase 1
            filler = [qkv_unit(1, 4), qkv_unit(1, 5),
                      vaug_unit(1, 2), vaug_unit(1, 3),
                      oxpose_unit(0),
                      late_const_dma_unit(),
                      qkv_unit(2, 2), qkv_unit(2, 3),
                      qkv_unit(2, 0), qkv_unit(2, 1),
                      vaug_unit(2, 0), vaug_unit(2, 1)]
            _interleave(attn_units(1), filler)
            # phase 2
            filler = [qkv_unit(2, 4), qkv_unit(2, 5),
                      vaug_unit(2, 2), vaug_unit(2, 3),
                      xdma_unit(3), oxpose_unit(1),
                      qkv_unit(3, 2), qkv_unit(3, 3),
                      oproj_unit(0, act_share=False),
                      qkv_unit(3, 0), qkv_unit(3, 1),
                      vaug_unit(3, 0), vaug_unit(3, 1),
                      oproj_unit(1, act_share=False)]
            _interleave(attn_units(2), filler)
            # phase 3 (ACT-bound): oproj(2..11) as PE filler, back-loaded.
            # oxpose_pe(3, 0) reads h0 AND h1 attO columns, so it must come
            # after flush(h1) — index 18 of the overlap-ordered a3 list.
            oxpose_unit(2)()
            a3 = attn_units(3)
            _interleave(a3[:19],
                        [qkv_unit(3, 4), oproj_unit(2, act_share=False),
                         qkv_unit(3, 5),
                         vaug_unit(3, 2), vaug_unit(3, 3)] +
                        [oproj_unit(tb, act_share=False)
                         for tb in range(3, 6)])
            a3b = [oxpose_pe_unit(3, 0)] + a3[19:]
            _interleave(a3b, [oproj_unit(tb, act_share=False)
                              for tb in range(6, 12)])
            # epilogue: pipeline the last 4 transposes+oprojs on freed psum
            oxpose_pe_unit(3, 1, ibs=(0,), tag="pv")()
            oxpose_pe_unit(3, 1, ibs=(1,), tag="pv")()
            oproj_unit(12, tail=True)()
            oxpose_pe_unit(3, 1, ibs=(2,), tag="pv")()
            oproj_unit(13, tail=True, tail_dve=True)()
            oxpose_pe_unit(3, 1, ibs=(3,), tag="pv")()
            oproj_unit(14, tail=True)()
            oproj_unit(15, tail=True, tail_dve=True)()
            if dbg:
                for nm, t in (("vT", vT),
                              ("attT", attT), ("vaug0", vaug[0]),
                              ("attO", attO)):
                    nc.sync.dma_start(dbg_d[nm][:], t[:])

    nc.compile()
    return nc


def _prep_inputs(x, qkv_w, qkv_b, o_w):
    """Per-core input maps (batch x head sharding), fp8+bf16 host-side prep."""
    import ml_dtypes
    bf16 = ml_dtypes.bfloat16
    fp8 = ml_dtypes.float8_e4m3
    fp8e5 = ml_dtypes.float8_e5m2

    x = np.asarray(x, dtype=np.float32)
    qkv_w = np.asarray(qkv_w, dtype=np.float32)
    qkv_b = np.asarray(qkv_b, dtype=np.float32)
    o_w = np.asarray(o_w, dtype=np.float32)
    tri = np.triu(np.ones((128, 128), dtype=np.float32)).astype(bf16)
    ident = np.eye(128, dtype=np.float32).astype(bf16)

    x8_b, xr8_b = [], []
    for b in range(B):
        xT = np.ascontiguousarray(x[b].T)
        x8 = xT.astype(fp8)
        x8f = x8.astype(np.float32)
        x8_b.append(x8)
        xr8_b.append((xT - x8f).astype(fp8))

    in_maps = []
    for c in range(NCORES):
        b = c // CPB
        lo = (c % CPB) * 256
        # m-tiles: q0,k0,v0,v1,q1,k1 (128 cols each) -> [128, (mt, kb, 128)]
        w_c = np.concatenate(
            [qkv_w[:, lo:lo + 128],
             qkv_w[:, C + lo:C + lo + 128],
             qkv_w[:, 2 * C + lo:2 * C + lo + 128],
             qkv_w[:, 2 * C + lo + 128:2 * C + lo + 256],
             qkv_w[:, lo + 128:lo + 256],
             qkv_w[:, C + lo + 128:C + lo + 256]], axis=1)   # [1024, 768]
        w_c = np.ascontiguousarray(
            w_c.reshape(KB, 128, 6, 128).transpose(1, 2, 0, 3)
            .reshape(128, 6 * KB * 128))
        w8_c = w_c.astype(fp8)
        r8_c = (w_c - w8_c.astype(np.float32)).astype(fp8e5)
        b_c = np.stack(
            [qkv_b[lo:lo + 128],
             qkv_b[C + lo:C + lo + 128],
             qkv_b[2 * C + lo:2 * C + lo + 128],
             qkv_b[2 * C + lo + 128:2 * C + lo + 256],
             qkv_b[lo + 128:lo + 256],
             qkv_b[C + lo + 128:C + lo + 256]], axis=1)  # [128, 6]
        ow_c = np.ascontiguousarray(
            o_w[lo:lo + 256, :].reshape(2, 128, C).transpose(1, 0, 2)
            .reshape(128, 2 * C)).astype(bf16)
        in_maps.append({
            "x8": x8_b[b],
            "xr8": xr8_b[b],
            "w": w8_c,
            "r": r8_c,
            "bqkv": np.ascontiguousarray(b_c, dtype=np.float32),
            "ow": ow_c,
            "tri": tri,
            "ident": ident,
        })
    return in_maps


def kernel(x, qkv_w, qkv_b, o_w, o_b):
    global _nc_cache
    from concourse import bass_utils
    if _nc_cache is None:
        _nc_cache = build_bass()
    nc = _nc_cache
    in_maps = _prep_inputs(x, qkv_w, qkv_b, o_w)
    res = bass_utils.run_bass_kernel_spmd(nc, in_maps, core_ids=list(range(NCORES)))
    o_b = np.asarray(o_b, dtype=np.float64)
    y = np.zeros((B, T, C), dtype=np.float64)
    for c in range(NCORES):
        y[c // CPB] += res.results[c]["y"].astype(np.float64)
    return (y + o_b[None, None, :]).astype(np.float32)

